# revision 49
# baseline (speedup 1.0000x reference)
"""Bass/Tile TRN2 kernel for nn_Attention_12704513261709 (low-rank factored).

Per-head dim (2048) >> model dim (256), so fold each head's weight pairs
into 256x256 matrices on the host:
  S_h = xn @ M_h @ xn^T    M_h = SCALE * diag(1+g) Wq_h^T Wk_h diag(1+g)
  Y_h = softmax(S_h) @ xn @ G_h    G_h = diag(1+g) Wv_h^T Wo_h^T
This cuts matmul FLOPs ~8.9x vs materializing q/k/v. Each of the 8 cores
computes one head over both batches; host sums the per-head partials.

Perf design. The PE p-state ramp (1.2 GHz until ~3us of continuous busy,
2.4 GHz after; stalls reset it) and the ~166ns non-overlapped SBUF access
latency paid by any matmul that carries a semaphore wait mean the matmul
stream must be both gap-free and wait-free:
 - xn transposes are XBAR DMA-transposes via a DRAM round-trip.
 - xn/xnT/tT live in per-512-token-group tiles so dependency tracking is
   group-granular (one whole-tensor tile would make early readers wait on
   all later writers).
 - S^T tiles are computed in pairs into [128,1024] 2-bank PSUM tiles, one
   exp per pair; U/Y of chunk q are deferred into chunk q+1's S-phase
   slots so the PE interleaves while ACT exps trail.
 - U consumes P^T k-slices in reverse production order: its first matmul
   waits the newest exp event and every later wait is elided as redundant.
 - softmax rowsum: contiguous bf16 add-ladder on DVE, then the partition-
   partial sums go to DRAM and the HOST finishes the reduction and the
   divide (device reciprocal + scale would sit on the DVE critical path).
 - LN sqrt/recip batched per 16 tiles (no ACT Sqrt/Exp table thrash).
"""

import numpy as np
import ml_dtypes

B = 2
N_SEQ = 2048
N_TOK = B * N_SEQ  # 4096
D = 256
HEADS = 8
INNER = 16384
DH = INNER // HEADS  # 2048
SCALE = 64 ** (-0.5)
EPS = 1e-5

TT = N_SEQ // 128  # 16 key tiles per batch
NCH = N_SEQ // 512  # 4 query chunks of 512 per batch
NG = N_TOK // 512  # 8 512-token groups
NPAIR = TT // 2  # 8 S-tile pairs per chunk

_CACHE = {}


def _build():
    from concourse import bacc, bass_isa
    import concourse.tile as tile
    import concourse.mybir as mybir

    f32 = mybir.dt.float32
    bf16 = mybir.dt.bfloat16
    AF = mybir.ActivationFunctionType
    ALU = mybir.AluOpType

    nc = bacc.Bacc("TRN2", target_bir_lowering=False, debug=False, num_devices=8)

    x_d = nc.dram_tensor("x", [N_TOK, D], f32, kind="ExternalInput").ap()
    m_d = nc.dram_tensor("m", [D, D], bf16, kind="ExternalInput").ap()
    g_d = nc.dram_tensor("g", [D, D], bf16, kind="ExternalInput").ap()
    out_d = nc.dram_tensor("outT", [D, N_TOK], f32, kind="ExternalOutput").ap()
    rsum_d = nc.dram_tensor("rsum", [NG, 128, 512], f32, kind="ExternalOutput").ap()

    with tile.TileContext(nc) as tc:
        with (
            tc.tile_pool(name="singles", bufs=1) as singles,
            tc.tile_pool(name="xt", bufs=6) as xt_pool,
            tc.tile_pool(name="lns", bufs=4) as lns_pool,
            tc.tile_pool(name="big", bufs=1) as big,
            tc.tile_pool(name="pt", bufs=2) as pt_pool,
            tc.tile_pool(name="ut", bufs=2) as ut_pool,
            tc.tile_pool(name="lad", bufs=1) as lad_pool,
            tc.tile_pool(name="rsum", bufs=2) as rsum_pool,
            tc.tile_pool(name="ystage", bufs=2) as y_pool,
            tc.tile_pool(name="dram", bufs=1, space="DRAM") as dram_pool,
            tc.tile_pool(name="psA", bufs=2, space="PSUM") as psA,
            tc.tile_pool(name="psUY", bufs=2, space="PSUM") as psUY,
        ):
            eps_t = singles.tile([128, 1], f32)
            nc.vector.memset(eps_t, EPS)
            dummy_w = singles.tile([128, 128], bf16)
            nc.vector.memset(dummy_w, 0.0)
            dummy_r = singles.tile([128, 512], bf16)
            nc.vector.memset(dummy_r, 0.0)

            def warm(n):
                for _ in range(n):
                    ps = psUY.tile([128, 1024], f32, tag="u", name="hamwarm")
                    nc.tensor.matmul(ps[:, :512], dummy_w[:], dummy_r[:], start=True, stop=True)

            m_sb = [big.tile([128, D], bf16, tag=f"m{c}", name=f"m{c}") for c in range(2)]
            g_sb = [big.tile([128, D], bf16, tag=f"g{c}", name=f"g{c}") for c in range(2)]
            # per-512-token-group tiles (group-granular dependency tracking)
            xng = [big.tile([128, 4 * D], bf16, tag=f"xng{g}", name=f"xng{g}") for g in range(NG)]
            # xnT in 1024-token half-batch tiles: 2 XBAR transpose-loads each
            xnTh = [big.tile([128, 2, 1024], bf16, tag=f"xnTh{h}", name=f"xnTh{h}") for h in range(4)]
            tTg = [big.tile([128, 2, 512], bf16, tag=f"tTg{g}", name=f"tTg{g}") for g in range(NG)]
            mv_all = big.tile([128, 32, 2], f32, tag="mv", name="mv")
            rstd_all = big.tile([128, 32], f32, tag="rstd", name="rstd")
            # per-half DRAM scratch (one tile would serialize each transpose-
            # load behind ALL stores via whole-tile dependency tracking)
            xnd = [dram_pool.tile([1024, D], bf16, tag=f"xnd{h}", name=f"xnd{h}") for h in range(4)]

            state = {}

            def load_x4(g):
                x4 = xt_pool.tile([128, 4, D], f32, tag="x4", name="x4")
                nc.sync.dma_start(
                    x4[:], x_d[g * 512 : (g + 1) * 512, :].rearrange("(t p) d -> p t d", p=128)
                )
                state[f"x4_{g}"] = x4

            def ln_stats(t):
                x_t = state[f"x4_{t // 4}"][:, t % 4, :]
                stats = lns_pool.tile([128, nc.vector.BN_STATS_DIM], f32, tag="st", name="st")
                nc.vector.bn_stats(stats[:], x_t)
                nc.vector.bn_aggr(mv_all[:, t, :], stats[:])

            def stats4(g):
                for t in range(4 * g, 4 * g + 4):
                    ln_stats(t)

            def ln_finish16(h):
                std16 = lns_pool.tile([128, 16], f32, tag="std16", name="std16")
                nc.scalar.activation(
                    std16[:], mv_all[:, h * 16 : (h + 1) * 16, 1], func=AF.Sqrt,
                    bias=eps_t[:], scale=1.0,
                )
                nc.vector.reciprocal(rstd_all[:, h * 16 : (h + 1) * 16], std16[:])

            def norm_store(g):
                """LN-normalize group g; store each tile to the DRAM scratch
                right after its norm (per-tile stores release the transpose-
                load's RAW dependency several us earlier than a group store)."""
                for t in range(4 * g, 4 * g + 4):
                    nc.vector.tensor_scalar(
                        xng[g][:, (t % 4) * D : (t % 4 + 1) * D],
                        state[f"x4_{g}"][:, t % 4, :],
                        scalar1=mv_all[:, t, 0:1],
                        scalar2=rstd_all[:, t : t + 1],
                        op0=ALU.subtract,
                        op1=ALU.mult,
                    )
                    row = (g % 2) * 512 + (t % 4) * 128
                    nc.sync.dma_start(
                        xnd[g // 2][row : row + 128, :],
                        xng[g][:, (t % 4) * D : (t % 4 + 1) * D],
                    )

            def tload_half(h):
                """XBAR transpose-load one 1024-token half back into xnTh[h]."""
                for c in range(2):
                    nc.sync.dma_start_transpose(
                        xnTh[h][:, c, :],
                        xnd[h][:, c * 128 : (c + 1) * 128],
                    )

            def tT_group(g):
                ps = psUY.tile([128, 1024], f32, tag="u", name="tT")
                off = (g % 2) * 512
                for c2 in range(2):
                    for c1 in range(2):
                        nc.tensor.matmul(
                            ps[:, c2 * 512 : (c2 + 1) * 512],
                            m_sb[c1][:, c2 * 128 : (c2 + 1) * 128],
                            xnTh[g // 2][:, c1, off : off + 512],
                            start=(c1 == 0),
                            stop=(c1 == 1),
                        )
                nc.vector.tensor_copy(tTg[g][:], ps.rearrange("p (c q) -> p c q", c=2))

            def phase_s(b, ch, extras=()):
                """S^T pairs + exp for one 512-query chunk; extras[i] runs
                after pair i."""
                pt_big = pt_pool.tile([128, TT * 512], bf16, tag="pt", name="pt")
                state["pt"] = pt_big
                tt = tTg[b * NCH + ch]
                for p in range(NPAIR):
                    ps = psA.tile([128, 1024], f32, tag="s", name="s")
                    for kk in range(2):
                        t = 2 * p + kk
                        tg = b * TT + t
                        for c in range(2):
                            nc.tensor.matmul(
                                ps[:, kk * 512 : (kk + 1) * 512],
                                xnTh[tg // 8][:, c, (tg % 8) * 128 : (tg % 8 + 1) * 128],
                                tt[:, c, :],
                                start=(c == 0),
                                stop=(c == 1),
                            )
                    nc.scalar.activation(
                        pt_big[:, p * 1024 : (p + 1) * 1024], ps[:], func=AF.Exp
                    )
                    if p < len(extras):
                        th = extras[p]
                        if th is not None:
                            th()
                for j in range(NPAIR, len(extras)):
                    th = extras[j]
                    if th is not None:
                        th()

            def phase_rsum(q):
                """Partition-partial softmax denominators: bf16 add-ladder on
                DVE, partial [128,512] straight to DRAM (host finishes)."""
                pt_big = state["pt"]
                r1 = lad_pool.tile([128, 4096], bf16, tag="r1", name="r1")
                nc.vector.tensor_tensor(r1[:], pt_big[:, :4096], pt_big[:, 4096:], ALU.add)
                r2 = lad_pool.tile([128, 2048], bf16, tag="r2", name="r2")
                nc.vector.tensor_tensor(r2[:], r1[:, :2048], r1[:, 2048:], ALU.add)
                r3 = lad_pool.tile([128, 1024], bf16, tag="r3", name="r3")
                nc.vector.tensor_tensor(r3[:], r2[:, :1024], r2[:, 1024:], ALU.add)
                r4 = rsum_pool.tile([128, 512], f32, tag="r4", name="r4")
                nc.vector.tensor_tensor(r4[:], r3[:, :512], r3[:, 512:], ALU.add)
                nc.gpsimd.dma_start(rsum_d[q], r4[:])

            def mku_segs(b, ch, pt_big):
                """Deferred U-phase: 8 PE segments of 4 matmuls (fine-grained
                so every S-pair slot of the next chunk gets PE filler while
                ACT exps trail). P^T k-slices are consumed newest-exp-first so
                only the first matmul carries a wait; the e0/e1 chains land in
                the two halves of one 2-bank PSUM tile, each copied out as
                soon as its chain stops."""
                holder = {}

                def seg(e, j):
                    def run():
                        if "ps" not in holder:
                            holder["ps"] = psUY.tile([128, 1024], f32, tag="u", name="u")
                        ps = holder["ps"]
                        for t in range(15 - 4 * j, 11 - 4 * j, -1):
                            nc.tensor.matmul(
                                ps[:, e * 512 : (e + 1) * 512],
                                xng[b * NCH + t // 4][:, (t % 4) * D + e * 128 : (t % 4) * D + (e + 1) * 128],
                                pt_big[:, t * 512 : (t + 1) * 512],
                                start=(t == TT - 1),
                                stop=(t == 0),
                            )
                        if j == 3:
                            ut = ut_pool.tile([128, 512], bf16, tag=f"ut{e}", name=f"ut{e}")
                            nc.vector.tensor_copy(ut[:], ps[:, e * 512 : (e + 1) * 512])
                            holder[f"ut{e}"] = ut
                    return run

                return [seg(e, j) for e in range(2) for j in range(4)], holder

            def mky(b, ch, holder):
                def run():
                    cols = b * N_SEQ + ch * 512
                    ps = psUY.tile([128, 1024], f32, tag="u", name="y")
                    for c2 in range(2):
                        for e in range(2):
                            nc.tensor.matmul(
                                ps[:, c2 * 512 : (c2 + 1) * 512],
                                g_sb[e][:, c2 * 128 : (c2 + 1) * 128],
                                holder[f"ut{e}"][:],
                                start=(e == 0),
                                stop=(e == 1),
                            )
                    y_sb = y_pool.tile([128, 1024], f32, tag="y", name="y")
                    nc.vector.tensor_copy(y_sb[:], ps[:])
                    nc.sync.dma_start(
                        out_d[0:256, cols : cols + 512].rearrange("(c p) q -> p c q", p=128),
                        y_sb.rearrange("p (c q) -> p c q", c=2),
                    )
                return run

            def mk(f, *a):
                return lambda: f(*a)

            # ---- prologue ----
            nc.gpsimd.dma_start(m_sb[0][:], m_d[0:128, :])
            nc.gpsimd.dma_start(m_sb[1][:], m_d[128:256, :])
            nc.gpsimd.dma_start(g_sb[0][:], g_d[0:128, :])
            nc.gpsimd.dma_start(g_sb[1][:], g_d[128:256, :])
            warm(74)

            for g in range(4):
                load_x4(g)
            for g in range(4):
                stats4(g)
            ln_finish16(0)
            norm_store(0)
            norm_store(1)
            tload_half(0)
            norm_store(2)
            norm_store(3)
            tload_half(1)
            tT_group(0)
            tT_group(1)
            tT_group(2)
            tT_group(3)
            warm(4)

            def nst_b1(g, h=None):
                norm_store(g)
                if h is not None:
                    tload_half(h)

            preps = {
                (0, 0): [mk(load_x4, 4), mk(load_x4, 5), mk(stats4, 4), mk(stats4, 5)],
                (0, 1): [mk(load_x4, 6), mk(load_x4, 7), mk(stats4, 6), mk(stats4, 7),
                         mk(ln_finish16, 1)],
                (0, 2): [mk(nst_b1, 4), mk(nst_b1, 5, 2)],
                (0, 3): [mk(nst_b1, 6), mk(nst_b1, 7, 3)],
            }

            # Y of chunk q runs at slot 0 of chunk q+2 (its inputs are then
            # long ready); U of chunk q fills the S-pair slots of chunk q+1.
            segs_prev = None
            y1 = y2 = None
            for b in range(B):
                for ch in range(NCH):
                    q = b * NCH + ch
                    prep = list(preps.get((b, ch), []))
                    extras = []
                    if y2 is not None:
                        extras.append(y2)
                    if 4 < q + 1 < NG:
                        extras.append(mk(tT_group, q + 1))
                    elif q + 1 == 4:
                        prep.append(mk(tT_group, 4))
                    if segs_prev is not None:
                        # interleave the first preps between early U segs so
                        # their DVE work runs clear of the boundary backlog
                        s = list(segs_prev)
                        extras.extend([s[0], prep[0] if prep else None,
                                       s[1], prep[1] if len(prep) > 1 else None])
                        extras.extend(s[2:])
                        extras.extend(prep[2:])
                    else:
                        extras.extend(prep)
                    phase_s(b, ch, extras)
                    if q < NG - 1:
                        phase_rsum(q)
                    segs_prev, holder = mku_segs(b, ch, state["pt"])
                    y2 = y1
                    y1 = mky(b, ch, holder)
            y2()
            for th in segs_prev:
                th()
            y1()
            phase_rsum(NG - 1)

    nc.compile()
    return nc


def get_nc():
    if "nc" not in _CACHE:
        _CACHE["nc"] = _build()
    return _CACHE["nc"]


def make_in_maps(x, gamma, Wq, Wk, Wv, Wo):
    bf = ml_dtypes.bfloat16
    gp = 1.0 + gamma.astype(np.float64)
    x_flat = np.ascontiguousarray(x.reshape(N_TOK, D).astype(np.float32))
    Wq = Wq.astype(np.float64)
    Wk = Wk.astype(np.float64)
    Wv = Wv.astype(np.float64)
    Wo = Wo.astype(np.float64)
    in_maps = []
    for h in range(HEADS):
        sl = slice(h * DH, (h + 1) * DH)
        M = SCALE * (gp[:, None] * Wq[sl].T) @ (Wk[sl] * gp[None, :])
        G = (gp[:, None] * Wv[sl].T) @ Wo[:, sl].T
        in_maps.append(
            {
                "x": x_flat,
                "m": np.ascontiguousarray(M.astype(bf)),
                "g": np.ascontiguousarray(G.astype(bf)),
            }
        )
    return in_maps


def kernel(x, gamma, Wq, Wk, Wv, Wo):
    from concourse import bass_utils

    x, gamma, Wq, Wk, Wv, Wo = (
        np.asarray(a) for a in (x, gamma, Wq, Wk, Wv, Wo)
    )
    nc = get_nc()
    in_maps = make_in_maps(x, gamma, Wq, Wk, Wv, Wo)
    res = bass_utils.run_bass_kernel_spmd(
        nc, in_maps, core_ids=list(range(HEADS))
    )
    acc = np.zeros((D, N_TOK), np.float32)
    for h in range(HEADS):
        rsum = np.asarray(res.results[h]["rsum"], np.float32).sum(axis=1).reshape(-1)
        acc += res.results[h]["outT"] / rsum[None, :]
    return np.ascontiguousarray(acc.T).reshape(B, N_SEQ, D).astype(np.float32)


# revision 53
# speedup vs baseline: 1.1588x; 1.1588x over previous
"""Bass/Tile TRN2 kernel for nn_Attention_12704513261709 (low-rank factored).

Per-head dim (2048) >> model dim (256), so fold each head's weight pairs
into 256x256 matrices on the host:
  S_h = xn @ M_h @ xn^T    M_h = SCALE * diag(1+g) Wq_h^T Wk_h diag(1+g)
  Y_h = softmax(S_h) @ xn @ G_h    G_h = diag(1+g) Wv_h^T Wo_h^T
This cuts matmul FLOPs ~8.9x vs materializing q/k/v. Each of the 8 cores
computes one head over both batches; host sums the per-head partials.

Perf design. The PE p-state ramp (1.2 GHz until ~3us of continuous busy,
2.4 GHz after; stalls reset it) and the ~166ns non-overlapped SBUF access
latency paid by any matmul that carries a semaphore wait mean the matmul
stream must be both gap-free and wait-free:
 - xn transposes are XBAR DMA-transposes via a DRAM round-trip.
 - xn/xnT/tT live in per-512-token-group tiles so dependency tracking is
   group-granular (one whole-tensor tile would make early readers wait on
   all later writers).
 - S^T tiles are computed in pairs into [128,1024] 2-bank PSUM tiles, one
   exp per pair; U/Y of chunk q are deferred into chunk q+1's S-phase
   slots so the PE interleaves while ACT exps trail.
 - U consumes P^T k-slices in reverse production order: its first matmul
   waits the newest exp event and every later wait is elided as redundant.
 - softmax rowsum: contiguous bf16 add-ladder on DVE, then the partition-
   partial sums go to DRAM and the HOST finishes the reduction and the
   divide (device reciprocal + scale would sit on the DVE critical path).
 - LN sqrt/recip batched per 16 tiles (no ACT Sqrt/Exp table thrash).
"""

import numpy as np
import ml_dtypes

B = 2
N_SEQ = 2048
N_TOK = B * N_SEQ  # 4096
D = 256
HEADS = 8
INNER = 16384
DH = INNER // HEADS  # 2048
SCALE = 64 ** (-0.5)
EPS = 1e-5

TT = N_SEQ // 128  # 16 key tiles per batch
NCH = N_SEQ // 512  # 4 query chunks of 512 per batch
NG = N_TOK // 512  # 8 512-token groups
NPAIR = TT // 2  # 8 S-tile pairs per chunk

_CACHE = {}


def _build():
    from concourse import bacc, bass_isa
    import concourse.tile as tile
    import concourse.mybir as mybir

    f32 = mybir.dt.float32
    bf16 = mybir.dt.bfloat16
    AF = mybir.ActivationFunctionType
    ALU = mybir.AluOpType

    nc = bacc.Bacc("TRN2", target_bir_lowering=False, debug=False, num_devices=8)

    x_d = nc.dram_tensor("x", [N_TOK, D], f32, kind="ExternalInput").ap()
    m_d = nc.dram_tensor("m", [D, D], bf16, kind="ExternalInput").ap()
    g_d = nc.dram_tensor("g", [D, D], bf16, kind="ExternalInput").ap()
    out_d = nc.dram_tensor("outT", [D, N_TOK], f32, kind="ExternalOutput").ap()
    rsum_d = nc.dram_tensor("rsum", [NG, 128, 512], f32, kind="ExternalOutput").ap()

    with tile.TileContext(nc) as tc:
        with (
            tc.tile_pool(name="singles", bufs=1) as singles,
            tc.tile_pool(name="xt", bufs=6) as xt_pool,
            tc.tile_pool(name="lns", bufs=4) as lns_pool,
            tc.tile_pool(name="big", bufs=1) as big,
            tc.tile_pool(name="pt", bufs=2) as pt_pool,
            tc.tile_pool(name="ut", bufs=2) as ut_pool,
            tc.tile_pool(name="lad", bufs=1) as lad_pool,
            tc.tile_pool(name="rsum", bufs=2) as rsum_pool,
            tc.tile_pool(name="ystage", bufs=2) as y_pool,
            tc.tile_pool(name="dram", bufs=1, space="DRAM") as dram_pool,
            tc.tile_pool(name="psA", bufs=2, space="PSUM") as psA,
            tc.tile_pool(name="psUY", bufs=2, space="PSUM") as psUY,
        ):
            eps_t = singles.tile([128, 1], f32)
            nc.vector.memset(eps_t, EPS)
            dummy_w = singles.tile([128, 128], bf16)
            nc.vector.memset(dummy_w, 0.0)
            dummy_r = singles.tile([128, 512], bf16)
            nc.vector.memset(dummy_r, 0.0)

            def warm(n):
                for _ in range(n):
                    ps = psUY.tile([128, 1024], f32, tag="u", name="hamwarm")
                    nc.tensor.matmul(ps[:, :512], dummy_w[:], dummy_r[:], start=True, stop=True)

            m_sb = [big.tile([128, D], bf16, tag=f"m{c}", name=f"m{c}") for c in range(2)]
            g_sb = [big.tile([128, D], bf16, tag=f"g{c}", name=f"g{c}") for c in range(2)]
            # per-512-token-group tiles (group-granular dependency tracking)
            xng = [big.tile([128, 4 * D], bf16, tag=f"xng{g}", name=f"xng{g}") for g in range(NG)]
            # xnT in 1024-token half-batch tiles: 2 XBAR transpose-loads each
            xnTh = [big.tile([128, 2, 1024], bf16, tag=f"xnTh{h}", name=f"xnTh{h}") for h in range(4)]
            tTg = [big.tile([128, 2, 512], bf16, tag=f"tTg{g}", name=f"tTg{g}") for g in range(NG)]
            mv_all = big.tile([128, 32, 2], f32, tag="mv", name="mv")
            rstd_all = big.tile([128, 32], f32, tag="rstd", name="rstd")
            # per-half DRAM scratch (one tile would serialize each transpose-
            # load behind ALL stores via whole-tile dependency tracking)
            xnd = [dram_pool.tile([1024, D], bf16, tag=f"xnd{h}", name=f"xnd{h}") for h in range(4)]

            state = {}

            def load_x4(g, split=False):
                x4 = xt_pool.tile([128, 4, D], f32, tag="x4", name="x4")
                if split:
                    # halve latency of the critical first slab via two queues
                    nc.sync.dma_start(
                        x4[:, 0:2, :],
                        x_d[g * 512 : g * 512 + 256, :].rearrange("(t p) d -> p t d", p=128),
                    )
                    nc.scalar.dma_start(
                        x4[:, 2:4, :],
                        x_d[g * 512 + 256 : (g + 1) * 512, :].rearrange("(t p) d -> p t d", p=128),
                    )
                else:
                    nc.sync.dma_start(
                        x4[:], x_d[g * 512 : (g + 1) * 512, :].rearrange("(t p) d -> p t d", p=128)
                    )
                state[f"x4_{g}"] = x4

            def ln_stats(t):
                x_t = state[f"x4_{t // 4}"][:, t % 4, :]
                stats = lns_pool.tile([128, nc.vector.BN_STATS_DIM], f32, tag="st", name="st")
                nc.vector.bn_stats(stats[:], x_t)
                nc.vector.bn_aggr(mv_all[:, t, :], stats[:])

            def stats4(g):
                for t in range(4 * g, 4 * g + 4):
                    ln_stats(t)

            def ln_finish8(g8):
                std8 = lns_pool.tile([128, 8], f32, tag="std8", name="std8")
                nc.scalar.activation(
                    std8[:], mv_all[:, g8 * 8 : (g8 + 1) * 8, 1], func=AF.Sqrt,
                    bias=eps_t[:], scale=1.0,
                )
                nc.vector.reciprocal(rstd_all[:, g8 * 8 : (g8 + 1) * 8], std8[:])

            def norm_store(g):
                """LN-normalize group g; store each tile to the DRAM scratch
                right after its norm (per-tile stores release the transpose-
                load's RAW dependency several us earlier than a group store)."""
                for t in range(4 * g, 4 * g + 4):
                    nc.vector.tensor_scalar(
                        xng[g][:, (t % 4) * D : (t % 4 + 1) * D],
                        state[f"x4_{g}"][:, t % 4, :],
                        scalar1=mv_all[:, t, 0:1],
                        scalar2=rstd_all[:, t : t + 1],
                        op0=ALU.subtract,
                        op1=ALU.mult,
                    )
                    row = (g % 2) * 512 + (t % 4) * 128
                    nc.sync.dma_start(
                        xnd[g // 2][row : row + 128, :],
                        xng[g][:, (t % 4) * D : (t % 4 + 1) * D],
                    )

            def tload_half(h):
                """XBAR transpose-load one 1024-token half back into xnTh[h]."""
                for c in range(2):
                    nc.sync.dma_start_transpose(
                        xnTh[h][:, c, :],
                        xnd[h][:, c * 128 : (c + 1) * 128],
                    )

            def tT_group(g):
                ps = psUY.tile([128, 1024], f32, tag="u", name="tT")
                off = (g % 2) * 512
                for c2 in range(2):
                    for c1 in range(2):
                        nc.tensor.matmul(
                            ps[:, c2 * 512 : (c2 + 1) * 512],
                            m_sb[c1][:, c2 * 128 : (c2 + 1) * 128],
                            xnTh[g // 2][:, c1, off : off + 512],
                            start=(c1 == 0),
                            stop=(c1 == 1),
                        )
                nc.vector.tensor_copy(tTg[g][:], ps.rearrange("p (c q) -> p c q", c=2))

            def phase_s(b, ch, extras=()):
                """S^T pairs + exp for one 512-query chunk; extras[i] runs
                after pair i."""
                pt_big = pt_pool.tile([128, TT * 512], bf16, tag="pt", name="pt")
                state["pt"] = pt_big
                tt = tTg[b * NCH + ch]
                for p in range(NPAIR):
                    ps = psA.tile([128, 1024], f32, tag="s", name="s")
                    for kk in range(2):
                        t = 2 * p + kk
                        tg = b * TT + t
                        for c in range(2):
                            nc.tensor.matmul(
                                ps[:, kk * 512 : (kk + 1) * 512],
                                xnTh[tg // 8][:, c, (tg % 8) * 128 : (tg % 8 + 1) * 128],
                                tt[:, c, :],
                                start=(c == 0),
                                stop=(c == 1),
                            )
                    nc.scalar.activation(
                        pt_big[:, p * 1024 : (p + 1) * 1024], ps[:], func=AF.Exp
                    )
                    if p < len(extras):
                        th = extras[p]
                        if th is not None:
                            th()
                for j in range(NPAIR, len(extras)):
                    th = extras[j]
                    if th is not None:
                        th()

            def phase_rsum(q):
                """Partition-partial softmax denominators: bf16 add-ladder on
                DVE, partial [128,512] straight to DRAM (host finishes)."""
                pt_big = state["pt"]
                r1 = lad_pool.tile([128, 4096], bf16, tag="r1", name="r1")
                nc.vector.tensor_tensor(r1[:], pt_big[:, :4096], pt_big[:, 4096:], ALU.add)
                r2 = lad_pool.tile([128, 2048], bf16, tag="r2", name="r2")
                nc.vector.tensor_tensor(r2[:], r1[:, :2048], r1[:, 2048:], ALU.add)
                r3 = lad_pool.tile([128, 1024], bf16, tag="r3", name="r3")
                nc.vector.tensor_tensor(r3[:], r2[:, :1024], r2[:, 1024:], ALU.add)
                r4 = rsum_pool.tile([128, 512], f32, tag="r4", name="r4")
                nc.vector.tensor_tensor(r4[:], r3[:, :512], r3[:, 512:], ALU.add)
                nc.gpsimd.dma_start(rsum_d[q], r4[:])

            def mku_segs(b, ch, pt_big):
                """Deferred U-phase: 8 PE segments of 4 matmuls (fine-grained
                so every S-pair slot of the next chunk gets PE filler while
                ACT exps trail). P^T k-slices are consumed newest-exp-first so
                only the first matmul carries a wait; the e0/e1 chains land in
                the two halves of one 2-bank PSUM tile, each copied out as
                soon as its chain stops."""
                holder = {}

                def seg(e, j):
                    def run():
                        if "ps" not in holder:
                            holder["ps"] = psUY.tile([128, 1024], f32, tag="u", name="u")
                        ps = holder["ps"]
                        for t in range(15 - 4 * j, 11 - 4 * j, -1):
                            nc.tensor.matmul(
                                ps[:, e * 512 : (e + 1) * 512],
                                xng[b * NCH + t // 4][:, (t % 4) * D + e * 128 : (t % 4) * D + (e + 1) * 128],
                                pt_big[:, t * 512 : (t + 1) * 512],
                                start=(t == TT - 1),
                                stop=(t == 0),
                            )
                        if j == 3:
                            ut = ut_pool.tile([128, 512], bf16, tag=f"ut{e}", name=f"ut{e}")
                            nc.vector.tensor_copy(ut[:], ps[:, e * 512 : (e + 1) * 512])
                            holder[f"ut{e}"] = ut
                    return run

                return [seg(e, j) for e in range(2) for j in range(4)], holder

            def mky(b, ch, holder):
                def run():
                    cols = b * N_SEQ + ch * 512
                    ps = psUY.tile([128, 1024], f32, tag="u", name="y")
                    for c2 in range(2):
                        for e in range(2):
                            nc.tensor.matmul(
                                ps[:, c2 * 512 : (c2 + 1) * 512],
                                g_sb[e][:, c2 * 128 : (c2 + 1) * 128],
                                holder[f"ut{e}"][:],
                                start=(e == 0),
                                stop=(e == 1),
                            )
                    y_sb = y_pool.tile([128, 1024], f32, tag="y", name="y")
                    nc.vector.tensor_copy(y_sb[:], ps[:])
                    nc.sync.dma_start(
                        out_d[0:256, cols : cols + 512].rearrange("(c p) q -> p c q", p=128),
                        y_sb.rearrange("p (c q) -> p c q", c=2),
                    )
                return run

            def mk(f, *a):
                return lambda: f(*a)

            # ---- prologue ----
            nc.gpsimd.dma_start(m_sb[0][:], m_d[0:128, :])
            nc.gpsimd.dma_start(m_sb[1][:], m_d[128:256, :])
            nc.gpsimd.dma_start(g_sb[0][:], g_d[0:128, :])
            nc.gpsimd.dma_start(g_sb[1][:], g_d[128:256, :])
            warm(74)

            load_x4(0, split=True)
            for g in range(1, 4):
                load_x4(g)
            stats4(0)
            stats4(1)
            ln_finish8(0)
            norm_store(0)
            norm_store(1)
            tload_half(0)
            stats4(2)
            stats4(3)
            ln_finish8(1)
            norm_store(2)
            norm_store(3)
            tload_half(1)
            tT_group(0)
            tT_group(1)
            tT_group(2)
            tT_group(3)
            warm(4)

            def nst_b1(g, h=None):
                norm_store(g)
                if h is not None:
                    tload_half(h)

            preps = {
                # chunk 0 has no deferred U yet: pad its S-pair slots with
                # dummy matmuls so the PE never outruns the ACT exp stream
                (0, 0): [mk(warm, 2), mk(load_x4, 4), mk(warm, 2), mk(load_x4, 5),
                         mk(warm, 2), mk(stats4, 4), mk(warm, 2), mk(stats4, 5)],
                (0, 1): [mk(load_x4, 6), mk(load_x4, 7), mk(stats4, 6), mk(stats4, 7),
                         mk(ln_finish8, 2), mk(ln_finish8, 3)],
                (0, 2): [mk(nst_b1, 4), mk(nst_b1, 5, 2)],
                (0, 3): [mk(nst_b1, 6), mk(nst_b1, 7, 3)],
            }

            # Y of chunk q runs at slot 0 of chunk q+2 (its inputs are then
            # long ready); U of chunk q fills the S-pair slots of chunk q+1.
            segs_prev = None
            y1 = y2 = None
            for b in range(B):
                for ch in range(NCH):
                    q = b * NCH + ch
                    prep = list(preps.get((b, ch), []))
                    extras = []
                    if y2 is not None:
                        extras.append(y2)
                    if 4 < q + 1 < NG:
                        extras.append(mk(tT_group, q + 1))
                    elif q + 1 == 4:
                        prep.append(mk(tT_group, 4))
                    if segs_prev is not None:
                        # interleave the first preps between early U segs so
                        # their DVE work runs clear of the boundary backlog
                        s = list(segs_prev)
                        extras.extend([s[0], prep[0] if prep else None,
                                       s[1], prep[1] if len(prep) > 1 else None])
                        extras.extend(s[2:])
                        extras.extend(prep[2:])
                    else:
                        extras.extend(prep)
                    phase_s(b, ch, extras)
                    if q < NG - 1:
                        phase_rsum(q)
                    segs_prev, holder = mku_segs(b, ch, state["pt"])
                    y2 = y1
                    y1 = mky(b, ch, holder)
            y2()
            for th in segs_prev:
                th()
            y1()
            phase_rsum(NG - 1)

    nc.compile()
    return nc


def get_nc():
    if "nc" not in _CACHE:
        _CACHE["nc"] = _build()
    return _CACHE["nc"]


def make_in_maps(x, gamma, Wq, Wk, Wv, Wo):
    bf = ml_dtypes.bfloat16
    gp = 1.0 + gamma.astype(np.float64)
    x_flat = np.ascontiguousarray(x.reshape(N_TOK, D).astype(np.float32))
    Wq = Wq.astype(np.float64)
    Wk = Wk.astype(np.float64)
    Wv = Wv.astype(np.float64)
    Wo = Wo.astype(np.float64)
    in_maps = []
    for h in range(HEADS):
        sl = slice(h * DH, (h + 1) * DH)
        M = SCALE * (gp[:, None] * Wq[sl].T) @ (Wk[sl] * gp[None, :])
        G = (gp[:, None] * Wv[sl].T) @ Wo[:, sl].T
        in_maps.append(
            {
                "x": x_flat,
                "m": np.ascontiguousarray(M.astype(bf)),
                "g": np.ascontiguousarray(G.astype(bf)),
            }
        )
    return in_maps


def kernel(x, gamma, Wq, Wk, Wv, Wo):
    from concourse import bass_utils

    x, gamma, Wq, Wk, Wv, Wo = (
        np.asarray(a) for a in (x, gamma, Wq, Wk, Wv, Wo)
    )
    nc = get_nc()
    in_maps = make_in_maps(x, gamma, Wq, Wk, Wv, Wo)
    res = bass_utils.run_bass_kernel_spmd(
        nc, in_maps, core_ids=list(range(HEADS))
    )
    acc = np.zeros((D, N_TOK), np.float32)
    for h in range(HEADS):
        rsum = np.asarray(res.results[h]["rsum"], np.float32).sum(axis=1).reshape(-1)
        acc += res.results[h]["outT"] / rsum[None, :]
    return np.ascontiguousarray(acc.T).reshape(B, N_SEQ, D).astype(np.float32)


# revision 54
# speedup vs baseline: 1.1628x; 1.0034x over previous
"""Bass/Tile TRN2 kernel for nn_Attention_12704513261709 (low-rank factored).

Per-head dim (2048) >> model dim (256), so fold each head's weight pairs
into 256x256 matrices on the host:
  S_h = xn @ M_h @ xn^T    M_h = SCALE * diag(1+g) Wq_h^T Wk_h diag(1+g)
  Y_h = softmax(S_h) @ xn @ G_h    G_h = diag(1+g) Wv_h^T Wo_h^T
This cuts matmul FLOPs ~8.9x vs materializing q/k/v. Each of the 8 cores
computes one head over both batches; host sums the per-head partials.

Perf design. The PE p-state ramp (1.2 GHz until ~3us of continuous busy,
2.4 GHz after; stalls reset it) and the ~166ns non-overlapped SBUF access
latency paid by any matmul that carries a semaphore wait mean the matmul
stream must be both gap-free and wait-free:
 - xn transposes are XBAR DMA-transposes via a DRAM round-trip.
 - xn/xnT/tT live in per-512-token-group tiles so dependency tracking is
   group-granular (one whole-tensor tile would make early readers wait on
   all later writers).
 - S^T tiles are computed in pairs into [128,1024] 2-bank PSUM tiles, one
   exp per pair; U/Y of chunk q are deferred into chunk q+1's S-phase
   slots so the PE interleaves while ACT exps trail.
 - U consumes P^T k-slices in reverse production order: its first matmul
   waits the newest exp event and every later wait is elided as redundant.
 - softmax rowsum: contiguous bf16 add-ladder on DVE, then the partition-
   partial sums go to DRAM and the HOST finishes the reduction and the
   divide (device reciprocal + scale would sit on the DVE critical path).
 - LN sqrt/recip batched per 16 tiles (no ACT Sqrt/Exp table thrash).
"""

import numpy as np
import ml_dtypes

B = 2
N_SEQ = 2048
N_TOK = B * N_SEQ  # 4096
D = 256
HEADS = 8
INNER = 16384
DH = INNER // HEADS  # 2048
SCALE = 64 ** (-0.5)
EPS = 1e-5

TT = N_SEQ // 128  # 16 key tiles per batch
NCH = N_SEQ // 512  # 4 query chunks of 512 per batch
NG = N_TOK // 512  # 8 512-token groups
NPAIR = TT // 2  # 8 S-tile pairs per chunk

_CACHE = {}


def _build():
    from concourse import bacc, bass_isa
    import concourse.tile as tile
    import concourse.mybir as mybir

    f32 = mybir.dt.float32
    bf16 = mybir.dt.bfloat16
    AF = mybir.ActivationFunctionType
    ALU = mybir.AluOpType

    nc = bacc.Bacc("TRN2", target_bir_lowering=False, debug=False, num_devices=8)

    x_d = nc.dram_tensor("x", [N_TOK, D], f32, kind="ExternalInput").ap()
    m_d = nc.dram_tensor("m", [D, D], bf16, kind="ExternalInput").ap()
    g_d = nc.dram_tensor("g", [D, D], bf16, kind="ExternalInput").ap()
    out_d = nc.dram_tensor("outT", [D, N_TOK], f32, kind="ExternalOutput").ap()
    rsum_d = nc.dram_tensor("rsum", [NG, 128, 512], f32, kind="ExternalOutput").ap()

    with tile.TileContext(nc) as tc:
        with (
            tc.tile_pool(name="singles", bufs=1) as singles,
            tc.tile_pool(name="xt", bufs=6) as xt_pool,
            tc.tile_pool(name="lns", bufs=4) as lns_pool,
            tc.tile_pool(name="big", bufs=1) as big,
            tc.tile_pool(name="pt", bufs=2) as pt_pool,
            tc.tile_pool(name="ut", bufs=2) as ut_pool,
            tc.tile_pool(name="lad", bufs=1) as lad_pool,
            tc.tile_pool(name="rsum", bufs=2) as rsum_pool,
            tc.tile_pool(name="ystage", bufs=2) as y_pool,
            tc.tile_pool(name="dram", bufs=1, space="DRAM") as dram_pool,
            tc.tile_pool(name="psA", bufs=2, space="PSUM") as psA,
            tc.tile_pool(name="psUY", bufs=2, space="PSUM") as psUY,
        ):
            eps_t = singles.tile([128, 1], f32)
            nc.vector.memset(eps_t, EPS)
            dummy_w = singles.tile([128, 128], bf16)
            nc.vector.memset(dummy_w, 0.0)
            dummy_r = singles.tile([128, 512], bf16)
            nc.vector.memset(dummy_r, 0.0)

            def warm(n):
                for _ in range(n):
                    ps = psUY.tile([128, 1024], f32, tag="u", name="hamwarm")
                    nc.tensor.matmul(ps[:, :512], dummy_w[:], dummy_r[:], start=True, stop=True)

            m_sb = [big.tile([128, D], bf16, tag=f"m{c}", name=f"m{c}") for c in range(2)]
            g_sb = [big.tile([128, D], bf16, tag=f"g{c}", name=f"g{c}") for c in range(2)]
            # per-512-token-group tiles (group-granular dependency tracking)
            xng = [big.tile([128, 4 * D], bf16, tag=f"xng{g}", name=f"xng{g}") for g in range(NG)]
            # xnT in 1024-token half-batch tiles: 2 XBAR transpose-loads each
            xnTh = [big.tile([128, 2, 1024], bf16, tag=f"xnTh{h}", name=f"xnTh{h}") for h in range(4)]
            tTg = [big.tile([128, 2, 512], bf16, tag=f"tTg{g}", name=f"tTg{g}") for g in range(NG)]
            mv_all = big.tile([128, 32, 2], f32, tag="mv", name="mv")
            rstd_all = big.tile([128, 32], f32, tag="rstd", name="rstd")
            # per-half DRAM scratch (one tile would serialize each transpose-
            # load behind ALL stores via whole-tile dependency tracking)
            xnd = [dram_pool.tile([1024, D], bf16, tag=f"xnd{h}", name=f"xnd{h}") for h in range(4)]

            state = {}

            def load_x4(g, split=False):
                x4 = xt_pool.tile([128, 4, D], f32, tag="x4", name="x4")
                if split:
                    # halve latency of the critical first slab via two queues
                    nc.sync.dma_start(
                        x4[:, 0:2, :],
                        x_d[g * 512 : g * 512 + 256, :].rearrange("(t p) d -> p t d", p=128),
                    )
                    nc.scalar.dma_start(
                        x4[:, 2:4, :],
                        x_d[g * 512 + 256 : (g + 1) * 512, :].rearrange("(t p) d -> p t d", p=128),
                    )
                else:
                    nc.sync.dma_start(
                        x4[:], x_d[g * 512 : (g + 1) * 512, :].rearrange("(t p) d -> p t d", p=128)
                    )
                state[f"x4_{g}"] = x4

            def ln_stats(t):
                x_t = state[f"x4_{t // 4}"][:, t % 4, :]
                stats = lns_pool.tile([128, nc.vector.BN_STATS_DIM], f32, tag="st", name="st")
                nc.vector.bn_stats(stats[:], x_t)
                nc.vector.bn_aggr(mv_all[:, t, :], stats[:])

            def stats4(g):
                for t in range(4 * g, 4 * g + 4):
                    ln_stats(t)

            def ln_finish8(g8):
                std8 = lns_pool.tile([128, 8], f32, tag="std8", name="std8")
                nc.scalar.activation(
                    std8[:], mv_all[:, g8 * 8 : (g8 + 1) * 8, 1], func=AF.Sqrt,
                    bias=eps_t[:], scale=1.0,
                )
                nc.vector.reciprocal(rstd_all[:, g8 * 8 : (g8 + 1) * 8], std8[:])

            def norm_store(g):
                """LN-normalize group g; store each tile to the DRAM scratch
                right after its norm (per-tile stores release the transpose-
                load's RAW dependency several us earlier than a group store)."""
                for t in range(4 * g, 4 * g + 4):
                    nc.vector.tensor_scalar(
                        xng[g][:, (t % 4) * D : (t % 4 + 1) * D],
                        state[f"x4_{g}"][:, t % 4, :],
                        scalar1=mv_all[:, t, 0:1],
                        scalar2=rstd_all[:, t : t + 1],
                        op0=ALU.subtract,
                        op1=ALU.mult,
                    )
                    row = (g % 2) * 512 + (t % 4) * 128
                    nc.sync.dma_start(
                        xnd[g // 2][row : row + 128, :],
                        xng[g][:, (t % 4) * D : (t % 4 + 1) * D],
                    )

            def tload_half(h):
                """XBAR transpose-load one 1024-token half back into xnTh[h]."""
                for c in range(2):
                    nc.sync.dma_start_transpose(
                        xnTh[h][:, c, :],
                        xnd[h][:, c * 128 : (c + 1) * 128],
                    )

            def tT_group(g):
                ps = psUY.tile([128, 1024], f32, tag="u", name="tT")
                off = (g % 2) * 512
                for c2 in range(2):
                    for c1 in range(2):
                        nc.tensor.matmul(
                            ps[:, c2 * 512 : (c2 + 1) * 512],
                            m_sb[c1][:, c2 * 128 : (c2 + 1) * 128],
                            xnTh[g // 2][:, c1, off : off + 512],
                            start=(c1 == 0),
                            stop=(c1 == 1),
                        )
                nc.vector.tensor_copy(tTg[g][:], ps.rearrange("p (c q) -> p c q", c=2))

            def phase_s(b, ch, extras=()):
                """S^T pairs + exp for one 512-query chunk; extras[i] runs
                after pair i."""
                pt_big = pt_pool.tile([128, TT * 512], bf16, tag="pt", name="pt")
                state["pt"] = pt_big
                tt = tTg[b * NCH + ch]
                for p in range(NPAIR):
                    ps = psA.tile([128, 1024], f32, tag="s", name="s")
                    for kk in range(2):
                        t = 2 * p + kk
                        tg = b * TT + t
                        for c in range(2):
                            nc.tensor.matmul(
                                ps[:, kk * 512 : (kk + 1) * 512],
                                xnTh[tg // 8][:, c, (tg % 8) * 128 : (tg % 8 + 1) * 128],
                                tt[:, c, :],
                                start=(c == 0),
                                stop=(c == 1),
                            )
                    nc.scalar.activation(
                        pt_big[:, p * 1024 : (p + 1) * 1024], ps[:], func=AF.Exp
                    )
                    if p < len(extras):
                        th = extras[p]
                        if th is not None:
                            th()
                for j in range(NPAIR, len(extras)):
                    th = extras[j]
                    if th is not None:
                        th()

            def phase_rsum(q):
                """Partition-partial softmax denominators: bf16 add-ladder on
                DVE, partial [128,512] straight to DRAM (host finishes)."""
                pt_big = state["pt"]
                r1 = lad_pool.tile([128, 4096], bf16, tag="r1", name="r1")
                nc.vector.tensor_tensor(r1[:], pt_big[:, :4096], pt_big[:, 4096:], ALU.add)
                r2 = lad_pool.tile([128, 2048], bf16, tag="r2", name="r2")
                nc.vector.tensor_tensor(r2[:], r1[:, :2048], r1[:, 2048:], ALU.add)
                r3 = lad_pool.tile([128, 1024], bf16, tag="r3", name="r3")
                nc.vector.tensor_tensor(r3[:], r2[:, :1024], r2[:, 1024:], ALU.add)
                r4 = rsum_pool.tile([128, 512], f32, tag="r4", name="r4")
                nc.vector.tensor_tensor(r4[:], r3[:, :512], r3[:, 512:], ALU.add)
                nc.gpsimd.dma_start(rsum_d[q], r4[:])

            def mku_segs(b, ch, pt_big):
                """Deferred U-phase: 8 PE segments of 4 matmuls (fine-grained
                so every S-pair slot of the next chunk gets PE filler while
                ACT exps trail). P^T k-slices are consumed newest-exp-first so
                only the first matmul carries a wait; the e0/e1 chains land in
                the two halves of one 2-bank PSUM tile, each copied out as
                soon as its chain stops."""
                holder = {}

                def seg(e, j):
                    def run():
                        if "ps" not in holder:
                            holder["ps"] = psUY.tile([128, 1024], f32, tag="u", name="u")
                        ps = holder["ps"]
                        for t in range(15 - 4 * j, 11 - 4 * j, -1):
                            nc.tensor.matmul(
                                ps[:, e * 512 : (e + 1) * 512],
                                xng[b * NCH + t // 4][:, (t % 4) * D + e * 128 : (t % 4) * D + (e + 1) * 128],
                                pt_big[:, t * 512 : (t + 1) * 512],
                                start=(t == TT - 1),
                                stop=(t == 0),
                            )
                        if j == 3:
                            ut = ut_pool.tile([128, 512], bf16, tag=f"ut{e}", name=f"ut{e}")
                            nc.vector.tensor_copy(ut[:], ps[:, e * 512 : (e + 1) * 512])
                            holder[f"ut{e}"] = ut
                    return run

                return [seg(e, j) for e in range(2) for j in range(4)], holder

            def mky(b, ch, holder):
                def run():
                    cols = b * N_SEQ + ch * 512
                    ps = psUY.tile([128, 1024], f32, tag="u", name="y")
                    for c2 in range(2):
                        for e in range(2):
                            nc.tensor.matmul(
                                ps[:, c2 * 512 : (c2 + 1) * 512],
                                g_sb[e][:, c2 * 128 : (c2 + 1) * 128],
                                holder[f"ut{e}"][:],
                                start=(e == 0),
                                stop=(e == 1),
                            )
                    y_sb = y_pool.tile([128, 1024], f32, tag="y", name="y")
                    nc.vector.tensor_copy(y_sb[:], ps[:])
                    nc.sync.dma_start(
                        out_d[0:256, cols : cols + 512].rearrange("(c p) q -> p c q", p=128),
                        y_sb.rearrange("p (c q) -> p c q", c=2),
                    )
                return run

            def mk(f, *a):
                return lambda: f(*a)

            # ---- prologue ----
            nc.gpsimd.dma_start(m_sb[0][:], m_d[0:128, :])
            nc.gpsimd.dma_start(m_sb[1][:], m_d[128:256, :])
            nc.gpsimd.dma_start(g_sb[0][:], g_d[0:128, :])
            nc.gpsimd.dma_start(g_sb[1][:], g_d[128:256, :])
            warm(74)

            for g in range(4):
                load_x4(g)
            for g in range(4):
                stats4(g)
            ln_finish8(0)
            ln_finish8(1)
            norm_store(0)
            norm_store(1)
            tload_half(0)
            norm_store(2)
            norm_store(3)
            tload_half(1)
            tT_group(0)
            tT_group(1)
            tT_group(2)
            tT_group(3)
            warm(4)

            def nst_b1(g, h=None):
                norm_store(g)
                if h is not None:
                    tload_half(h)

            preps = {
                # chunk 0 has no deferred U yet: pad its S-pair slots with
                # dummy matmuls so the PE never outruns the ACT exp stream
                (0, 0): [mk(warm, 2), mk(load_x4, 4), mk(warm, 2), mk(load_x4, 5),
                         mk(warm, 2), mk(stats4, 4), mk(warm, 2), mk(stats4, 5)],
                (0, 1): [mk(load_x4, 6), mk(load_x4, 7), mk(stats4, 6), mk(stats4, 7),
                         mk(ln_finish8, 2), mk(ln_finish8, 3)],
                (0, 2): [mk(nst_b1, 4), mk(nst_b1, 5, 2)],
                (0, 3): [mk(nst_b1, 6), mk(nst_b1, 7, 3)],
            }

            # Y of chunk q runs at slot 0 of chunk q+2 (its inputs are then
            # long ready); U of chunk q fills the S-pair slots of chunk q+1.
            segs_prev = None
            y1 = y2 = None
            for b in range(B):
                for ch in range(NCH):
                    q = b * NCH + ch
                    prep = list(preps.get((b, ch), []))
                    extras = []
                    if y2 is not None:
                        extras.append(y2)
                    if 4 < q + 1 < NG:
                        extras.append(mk(tT_group, q + 1))
                    elif q + 1 == 4:
                        prep.append(mk(tT_group, 4))
                    if segs_prev is not None:
                        # interleave the first preps between early U segs so
                        # their DVE work runs clear of the boundary backlog
                        s = list(segs_prev)
                        extras.extend([s[0], prep[0] if prep else None,
                                       s[1], prep[1] if len(prep) > 1 else None])
                        extras.extend(s[2:])
                        extras.extend(prep[2:])
                    else:
                        extras.extend(prep)
                    phase_s(b, ch, extras)
                    if q < NG - 1:
                        phase_rsum(q)
                    segs_prev, holder = mku_segs(b, ch, state["pt"])
                    y2 = y1
                    y1 = mky(b, ch, holder)
            y2()
            for th in segs_prev:
                th()
            y1()
            phase_rsum(NG - 1)

    nc.compile()
    return nc


def get_nc():
    if "nc" not in _CACHE:
        _CACHE["nc"] = _build()
    return _CACHE["nc"]


def make_in_maps(x, gamma, Wq, Wk, Wv, Wo):
    bf = ml_dtypes.bfloat16
    gp = 1.0 + gamma.astype(np.float64)
    x_flat = np.ascontiguousarray(x.reshape(N_TOK, D).astype(np.float32))
    Wq = Wq.astype(np.float64)
    Wk = Wk.astype(np.float64)
    Wv = Wv.astype(np.float64)
    Wo = Wo.astype(np.float64)
    in_maps = []
    for h in range(HEADS):
        sl = slice(h * DH, (h + 1) * DH)
        M = SCALE * (gp[:, None] * Wq[sl].T) @ (Wk[sl] * gp[None, :])
        G = (gp[:, None] * Wv[sl].T) @ Wo[:, sl].T
        in_maps.append(
            {
                "x": x_flat,
                "m": np.ascontiguousarray(M.astype(bf)),
                "g": np.ascontiguousarray(G.astype(bf)),
            }
        )
    return in_maps


def kernel(x, gamma, Wq, Wk, Wv, Wo):
    from concourse import bass_utils

    x, gamma, Wq, Wk, Wv, Wo = (
        np.asarray(a) for a in (x, gamma, Wq, Wk, Wv, Wo)
    )
    nc = get_nc()
    in_maps = make_in_maps(x, gamma, Wq, Wk, Wv, Wo)
    res = bass_utils.run_bass_kernel_spmd(
        nc, in_maps, core_ids=list(range(HEADS))
    )
    acc = np.zeros((D, N_TOK), np.float32)
    for h in range(HEADS):
        rsum = np.asarray(res.results[h]["rsum"], np.float32).sum(axis=1).reshape(-1)
        acc += res.results[h]["outT"] / rsum[None, :]
    return np.ascontiguousarray(acc.T).reshape(B, N_SEQ, D).astype(np.float32)


# revision 55
# speedup vs baseline: 1.2043x; 1.0358x over previous
"""Bass/Tile TRN2 kernel for nn_Attention_12704513261709 (low-rank factored).

Per-head dim (2048) >> model dim (256), so fold each head's weight pairs
into 256x256 matrices on the host:
  S_h = xn @ M_h @ xn^T    M_h = SCALE * diag(1+g) Wq_h^T Wk_h diag(1+g)
  Y_h = softmax(S_h) @ xn @ G_h    G_h = diag(1+g) Wv_h^T Wo_h^T
This cuts matmul FLOPs ~8.9x vs materializing q/k/v. Each of the 8 cores
computes one head over both batches; host sums the per-head partials.

Perf design. The PE p-state ramp (1.2 GHz until ~3us of continuous busy,
2.4 GHz after; stalls reset it) and the ~166ns non-overlapped SBUF access
latency paid by any matmul that carries a semaphore wait mean the matmul
stream must be both gap-free and wait-free:
 - xn transposes are XBAR DMA-transposes via a DRAM round-trip.
 - xn/xnT/tT live in per-512-token-group tiles so dependency tracking is
   group-granular (one whole-tensor tile would make early readers wait on
   all later writers).
 - S^T tiles are computed in pairs into [128,1024] 2-bank PSUM tiles, one
   exp per pair; U/Y of chunk q are deferred into chunk q+1's S-phase
   slots so the PE interleaves while ACT exps trail.
 - U consumes P^T k-slices in reverse production order: its first matmul
   waits the newest exp event and every later wait is elided as redundant.
 - softmax rowsum: contiguous bf16 add-ladder on DVE, then the partition-
   partial sums go to DRAM and the HOST finishes the reduction and the
   divide (device reciprocal + scale would sit on the DVE critical path).
 - LN sqrt/recip batched per 16 tiles (no ACT Sqrt/Exp table thrash).
"""

import numpy as np
import ml_dtypes

B = 2
N_SEQ = 2048
N_TOK = B * N_SEQ  # 4096
D = 256
HEADS = 8
INNER = 16384
DH = INNER // HEADS  # 2048
SCALE = 64 ** (-0.5)
EPS = 1e-5

TT = N_SEQ // 128  # 16 key tiles per batch
NCH = N_SEQ // 512  # 4 query chunks of 512 per batch
NG = N_TOK // 512  # 8 512-token groups
NPAIR = TT // 2  # 8 S-tile pairs per chunk

_CACHE = {}


def _build():
    from concourse import bacc, bass_isa
    import concourse.tile as tile
    import concourse.mybir as mybir

    f32 = mybir.dt.float32
    bf16 = mybir.dt.bfloat16
    AF = mybir.ActivationFunctionType
    ALU = mybir.AluOpType

    nc = bacc.Bacc("TRN2", target_bir_lowering=False, debug=False, num_devices=8)

    x_d = nc.dram_tensor("x", [N_TOK, D], f32, kind="ExternalInput").ap()
    m_d = nc.dram_tensor("m", [D, D], bf16, kind="ExternalInput").ap()
    g_d = nc.dram_tensor("g", [D, D], bf16, kind="ExternalInput").ap()
    out_d = nc.dram_tensor("outT", [D, N_TOK], f32, kind="ExternalOutput").ap()
    rsum_d = nc.dram_tensor("rsum", [NG, 128, 512], f32, kind="ExternalOutput").ap()

    with tile.TileContext(nc) as tc:
        with (
            tc.tile_pool(name="singles", bufs=1) as singles,
            tc.tile_pool(name="xt", bufs=6) as xt_pool,
            tc.tile_pool(name="lns", bufs=4) as lns_pool,
            tc.tile_pool(name="big", bufs=1) as big,
            tc.tile_pool(name="pt", bufs=2) as pt_pool,
            tc.tile_pool(name="ut", bufs=2) as ut_pool,
            tc.tile_pool(name="lad", bufs=1) as lad_pool,
            tc.tile_pool(name="rsum", bufs=2) as rsum_pool,
            tc.tile_pool(name="ystage", bufs=2) as y_pool,
            tc.tile_pool(name="dram", bufs=1, space="DRAM") as dram_pool,
            tc.tile_pool(name="psA", bufs=2, space="PSUM") as psA,
            tc.tile_pool(name="psUY", bufs=2, space="PSUM") as psUY,
        ):
            eps_t = singles.tile([128, 1], f32)
            nc.vector.memset(eps_t, EPS)
            dummy_w = singles.tile([128, 128], bf16)
            nc.vector.memset(dummy_w, 0.0)
            dummy_r = singles.tile([128, 512], bf16)
            nc.vector.memset(dummy_r, 0.0)

            def warm(n):
                for _ in range(n):
                    ps = psUY.tile([128, 1024], f32, tag="u", name="hamwarm")
                    nc.tensor.matmul(ps[:, :512], dummy_w[:], dummy_r[:], start=True, stop=True)

            m_sb = [big.tile([128, D], bf16, tag=f"m{c}", name=f"m{c}") for c in range(2)]
            g_sb = [big.tile([128, D], bf16, tag=f"g{c}", name=f"g{c}") for c in range(2)]
            # per-512-token-group tiles (group-granular dependency tracking)
            xng = [big.tile([128, 4 * D], bf16, tag=f"xng{g}", name=f"xng{g}") for g in range(NG)]
            # xnT in 1024-token half-batch tiles: 2 XBAR transpose-loads each
            xnTh = [big.tile([128, 2, 1024], bf16, tag=f"xnTh{h}", name=f"xnTh{h}") for h in range(4)]
            tTg = [big.tile([128, 2, 512], bf16, tag=f"tTg{g}", name=f"tTg{g}") for g in range(NG)]
            mv_all = big.tile([128, 32, 2], f32, tag="mv", name="mv")
            rstd_all = big.tile([128, 32], f32, tag="rstd", name="rstd")
            # per-half DRAM scratch (one tile would serialize each transpose-
            # load behind ALL stores via whole-tile dependency tracking)
            xnd = [dram_pool.tile([1024, D], bf16, tag=f"xnd{h}", name=f"xnd{h}") for h in range(4)]

            state = {}

            def load_x4(g, split=False):
                x4 = xt_pool.tile([128, 4, D], f32, tag="x4", name="x4")
                if split:
                    # halve latency of the critical first slab via two queues
                    nc.sync.dma_start(
                        x4[:, 0:2, :],
                        x_d[g * 512 : g * 512 + 256, :].rearrange("(t p) d -> p t d", p=128),
                    )
                    nc.scalar.dma_start(
                        x4[:, 2:4, :],
                        x_d[g * 512 + 256 : (g + 1) * 512, :].rearrange("(t p) d -> p t d", p=128),
                    )
                else:
                    nc.sync.dma_start(
                        x4[:], x_d[g * 512 : (g + 1) * 512, :].rearrange("(t p) d -> p t d", p=128)
                    )
                state[f"x4_{g}"] = x4

            def ln_stats(t):
                x_t = state[f"x4_{t // 4}"][:, t % 4, :]
                stats = lns_pool.tile([128, nc.vector.BN_STATS_DIM], f32, tag="st", name="st")
                nc.vector.bn_stats(stats[:], x_t)
                nc.vector.bn_aggr(mv_all[:, t, :], stats[:])

            def stats4(g):
                for t in range(4 * g, 4 * g + 4):
                    ln_stats(t)

            def ln_finish8(g8):
                std8 = lns_pool.tile([128, 8], f32, tag="std8", name="std8")
                nc.scalar.activation(
                    std8[:], mv_all[:, g8 * 8 : (g8 + 1) * 8, 1], func=AF.Sqrt,
                    bias=eps_t[:], scale=1.0,
                )
                nc.vector.reciprocal(rstd_all[:, g8 * 8 : (g8 + 1) * 8], std8[:])

            def norm_store(g):
                """LN-normalize group g and store it to the DRAM scratch."""
                for t in range(4 * g, 4 * g + 4):
                    nc.vector.tensor_scalar(
                        xng[g][:, (t % 4) * D : (t % 4 + 1) * D],
                        state[f"x4_{g}"][:, t % 4, :],
                        scalar1=mv_all[:, t, 0:1],
                        scalar2=rstd_all[:, t : t + 1],
                        op0=ALU.subtract,
                        op1=ALU.mult,
                    )
                nc.sync.dma_start(
                    xnd[g // 2][(g % 2) * 512 : (g % 2) * 512 + 512, :].rearrange(
                        "(t p) d -> p t d", p=128
                    ),
                    xng[g].rearrange("p (t d) -> p t d", t=4),
                )

            def tload_half(h):
                """XBAR transpose-load one 1024-token half back into xnTh[h]."""
                for c in range(2):
                    nc.sync.dma_start_transpose(
                        xnTh[h][:, c, :],
                        xnd[h][:, c * 128 : (c + 1) * 128],
                    )

            def tT_group(g):
                ps = psUY.tile([128, 1024], f32, tag="u", name="tT")
                off = (g % 2) * 512
                for c2 in range(2):
                    for c1 in range(2):
                        nc.tensor.matmul(
                            ps[:, c2 * 512 : (c2 + 1) * 512],
                            m_sb[c1][:, c2 * 128 : (c2 + 1) * 128],
                            xnTh[g // 2][:, c1, off : off + 512],
                            start=(c1 == 0),
                            stop=(c1 == 1),
                        )
                nc.vector.tensor_copy(tTg[g][:], ps.rearrange("p (c q) -> p c q", c=2))

            def phase_s(b, ch, extras=()):
                """S^T pairs + exp for one 512-query chunk; extras[i] runs
                after pair i."""
                pt_big = pt_pool.tile([128, TT * 512], bf16, tag="pt", name="pt")
                state["pt"] = pt_big
                tt = tTg[b * NCH + ch]
                for p in range(NPAIR):
                    ps = psA.tile([128, 1024], f32, tag="s", name="s")
                    for kk in range(2):
                        t = 2 * p + kk
                        tg = b * TT + t
                        for c in range(2):
                            nc.tensor.matmul(
                                ps[:, kk * 512 : (kk + 1) * 512],
                                xnTh[tg // 8][:, c, (tg % 8) * 128 : (tg % 8 + 1) * 128],
                                tt[:, c, :],
                                start=(c == 0),
                                stop=(c == 1),
                            )
                    nc.scalar.activation(
                        pt_big[:, p * 1024 : (p + 1) * 1024], ps[:], func=AF.Exp
                    )
                    if p < len(extras):
                        th = extras[p]
                        if th is not None:
                            th()
                for j in range(NPAIR, len(extras)):
                    th = extras[j]
                    if th is not None:
                        th()

            def phase_rsum(q):
                """Partition-partial softmax denominators: bf16 add-ladder on
                DVE, partial [128,512] straight to DRAM (host finishes)."""
                pt_big = state["pt"]
                r1 = lad_pool.tile([128, 4096], bf16, tag="r1", name="r1")
                nc.vector.tensor_tensor(r1[:], pt_big[:, :4096], pt_big[:, 4096:], ALU.add)
                r2 = lad_pool.tile([128, 2048], bf16, tag="r2", name="r2")
                nc.vector.tensor_tensor(r2[:], r1[:, :2048], r1[:, 2048:], ALU.add)
                r3 = lad_pool.tile([128, 1024], bf16, tag="r3", name="r3")
                nc.vector.tensor_tensor(r3[:], r2[:, :1024], r2[:, 1024:], ALU.add)
                r4 = rsum_pool.tile([128, 512], f32, tag="r4", name="r4")
                nc.vector.tensor_tensor(r4[:], r3[:, :512], r3[:, 512:], ALU.add)
                nc.gpsimd.dma_start(rsum_d[q], r4[:])

            def mku_segs(b, ch, pt_big):
                """Deferred U-phase: 8 PE segments of 4 matmuls (fine-grained
                so every S-pair slot of the next chunk gets PE filler while
                ACT exps trail). P^T k-slices are consumed newest-exp-first so
                only the first matmul carries a wait; the e0/e1 chains land in
                the two halves of one 2-bank PSUM tile, each copied out as
                soon as its chain stops."""
                holder = {}

                def seg(e, j):
                    def run():
                        if "ps" not in holder:
                            holder["ps"] = psUY.tile([128, 1024], f32, tag="u", name="u")
                        ps = holder["ps"]
                        for t in range(15 - 4 * j, 11 - 4 * j, -1):
                            nc.tensor.matmul(
                                ps[:, e * 512 : (e + 1) * 512],
                                xng[b * NCH + t // 4][:, (t % 4) * D + e * 128 : (t % 4) * D + (e + 1) * 128],
                                pt_big[:, t * 512 : (t + 1) * 512],
                                start=(t == TT - 1),
                                stop=(t == 0),
                            )
                        if j == 3:
                            ut = ut_pool.tile([128, 512], bf16, tag=f"ut{e}", name=f"ut{e}")
                            nc.vector.tensor_copy(ut[:], ps[:, e * 512 : (e + 1) * 512])
                            holder[f"ut{e}"] = ut
                    return run

                return [seg(e, j) for e in range(2) for j in range(4)], holder

            def mky(b, ch, holder):
                def run():
                    cols = b * N_SEQ + ch * 512
                    ps = psUY.tile([128, 1024], f32, tag="u", name="y")
                    for c2 in range(2):
                        for e in range(2):
                            nc.tensor.matmul(
                                ps[:, c2 * 512 : (c2 + 1) * 512],
                                g_sb[e][:, c2 * 128 : (c2 + 1) * 128],
                                holder[f"ut{e}"][:],
                                start=(e == 0),
                                stop=(e == 1),
                            )
                    y_sb = y_pool.tile([128, 1024], f32, tag="y", name="y")
                    nc.vector.tensor_copy(y_sb[:], ps[:])
                    nc.sync.dma_start(
                        out_d[0:256, cols : cols + 512].rearrange("(c p) q -> p c q", p=128),
                        y_sb.rearrange("p (c q) -> p c q", c=2),
                    )
                return run

            def mk(f, *a):
                return lambda: f(*a)

            # ---- prologue ----
            nc.gpsimd.dma_start(m_sb[0][:], m_d[0:128, :])
            nc.gpsimd.dma_start(m_sb[1][:], m_d[128:256, :])
            nc.gpsimd.dma_start(g_sb[0][:], g_d[0:128, :])
            nc.gpsimd.dma_start(g_sb[1][:], g_d[128:256, :])
            warm(74)

            for g in range(4):
                load_x4(g)
            for g in range(4):
                stats4(g)
            ln_finish8(0)
            ln_finish8(1)
            norm_store(0)
            norm_store(1)
            tload_half(0)
            norm_store(2)
            norm_store(3)
            tload_half(1)
            tT_group(0)
            tT_group(1)
            tT_group(2)
            tT_group(3)
            warm(4)

            def nst_b1(g, h=None):
                norm_store(g)
                if h is not None:
                    tload_half(h)

            preps = {
                # chunk 0 has no deferred U yet: pad its S-pair slots with
                # dummy matmuls so the PE never outruns the ACT exp stream
                (0, 0): [mk(warm, 2), mk(load_x4, 4), mk(warm, 2), mk(load_x4, 5),
                         mk(warm, 2), mk(stats4, 4), mk(warm, 2), mk(stats4, 5)],
                (0, 1): [mk(load_x4, 6), mk(load_x4, 7), mk(stats4, 6), mk(stats4, 7),
                         mk(ln_finish8, 2), mk(ln_finish8, 3)],
                (0, 2): [mk(nst_b1, 4), mk(nst_b1, 5, 2)],
                (0, 3): [mk(nst_b1, 6), mk(nst_b1, 7, 3)],
            }

            # Y of chunk q runs at slot 0 of chunk q+2 (its inputs are then
            # long ready); U of chunk q fills the S-pair slots of chunk q+1.
            segs_prev = None
            y1 = y2 = None
            for b in range(B):
                for ch in range(NCH):
                    q = b * NCH + ch
                    prep = list(preps.get((b, ch), []))
                    extras = []
                    if y2 is not None:
                        extras.append(y2)
                    if 4 < q + 1 < NG:
                        extras.append(mk(tT_group, q + 1))
                    elif q + 1 == 4:
                        prep.append(mk(tT_group, 4))
                    if segs_prev is not None:
                        # interleave the first preps between early U segs so
                        # their DVE work runs clear of the boundary backlog
                        s = list(segs_prev)
                        extras.extend([s[0], prep[0] if prep else None,
                                       s[1], prep[1] if len(prep) > 1 else None])
                        extras.extend(s[2:])
                        extras.extend(prep[2:])
                    else:
                        extras.extend(prep)
                    phase_s(b, ch, extras)
                    if q < NG - 1:
                        phase_rsum(q)
                    segs_prev, holder = mku_segs(b, ch, state["pt"])
                    y2 = y1
                    y1 = mky(b, ch, holder)
            y2()
            for th in segs_prev:
                th()
            y1()
            phase_rsum(NG - 1)

    nc.compile()
    return nc


def get_nc():
    if "nc" not in _CACHE:
        _CACHE["nc"] = _build()
    return _CACHE["nc"]


def make_in_maps(x, gamma, Wq, Wk, Wv, Wo):
    bf = ml_dtypes.bfloat16
    gp = 1.0 + gamma.astype(np.float64)
    x_flat = np.ascontiguousarray(x.reshape(N_TOK, D).astype(np.float32))
    Wq = Wq.astype(np.float64)
    Wk = Wk.astype(np.float64)
    Wv = Wv.astype(np.float64)
    Wo = Wo.astype(np.float64)
    in_maps = []
    for h in range(HEADS):
        sl = slice(h * DH, (h + 1) * DH)
        M = SCALE * (gp[:, None] * Wq[sl].T) @ (Wk[sl] * gp[None, :])
        G = (gp[:, None] * Wv[sl].T) @ Wo[:, sl].T
        in_maps.append(
            {
                "x": x_flat,
                "m": np.ascontiguousarray(M.astype(bf)),
                "g": np.ascontiguousarray(G.astype(bf)),
            }
        )
    return in_maps


def kernel(x, gamma, Wq, Wk, Wv, Wo):
    from concourse import bass_utils

    x, gamma, Wq, Wk, Wv, Wo = (
        np.asarray(a) for a in (x, gamma, Wq, Wk, Wv, Wo)
    )
    nc = get_nc()
    in_maps = make_in_maps(x, gamma, Wq, Wk, Wv, Wo)
    res = bass_utils.run_bass_kernel_spmd(
        nc, in_maps, core_ids=list(range(HEADS))
    )
    acc = np.zeros((D, N_TOK), np.float32)
    for h in range(HEADS):
        rsum = np.asarray(res.results[h]["rsum"], np.float32).sum(axis=1).reshape(-1)
        acc += res.results[h]["outT"] / rsum[None, :]
    return np.ascontiguousarray(acc.T).reshape(B, N_SEQ, D).astype(np.float32)
